# revision 11
# baseline (speedup 1.0000x reference)
"""Trainium2 Bass kernel v5 for nn_Block_39513699123558 (gnn_message_passing).

Two layers of (Chebyshev graph conv K=5 -> BatchNorm -> ReLU) on
x[B=2, F0=16, V=162, X=Y=Z=16].

v5: vertex-major bridge DRAM layout at 2-chunk (s=256) granularity.
  brS rows = (k, ch-out), cols = (padded vertex 168, s 256).  leg1 is
  ONE scatter DMA per T-stack span (512B runs); leg2 is one contiguous
  2KB-run DMA per projection u-slot into a tiny [*, 8KB] slot tile.
  Layer-1 k=0 rows are pre-staged by the host inside the bridge
  parameter; layer-2 k=0 reads the activated y-slab in place.
  Inputs and the h-bridge load at 2-chunk granularity (512B runs).
  Cheb contraction split 126+36 so band 3 is one clean piece.
"""

import os
import sys

sys.path.insert(0, "/opt/trn_rl_repo")

SKIP_CC = os.environ.get("K_SKIP_CC", "0") == "1"

import numpy as np
import ml_dtypes

from concourse import bass, bacc, mybir
from concourse import tile
from concourse.bass_utils import run_bass_kernel_spmd

BF16 = ml_dtypes.bfloat16
BF = mybir.dt.bfloat16
F32 = mybir.dt.float32

V = 162
VA = 126
VB = V - VA  # 36
F1, F2 = 16, 32
K = 5
S = 1024          # s-columns per core
SC = 128          # compute s-chunk
SB = 256          # bridge s-granularity (2 chunks)
NG = S // SB      # 4 groups
UP = 168          # padded vertex cols (4 bands x 42)
EPS = 1e-5
N_CORES = 8

UB = [0, 42, 84, 126, 162]
BW = [42, 42, 42, 36]

# T-stack k=1..4, 648 rows, 6 tiles of 108; spans (k, t, r0, u0, du)
SP1 = [(1, 0, 0, 0, 108), (1, 1, 0, 108, 54),
       (2, 1, 54, 0, 54), (2, 2, 0, 54, 108),
       (3, 3, 0, 0, 108), (3, 4, 0, 108, 54),
       (4, 4, 54, 0, 54), (4, 5, 0, 54, 108)]
SPANS_BY_T = {}
for _sp in SP1:
    SPANS_BY_T.setdefault(_sp[1], []).append(_sp)

NSLOT = 112
STSCR_W = NSLOT * 8
USLOTS = [(0, 4), (4, 4), (8, 4), (12, 4), (16, 4), (20, 4),
          (24, 4), (28, 4), (32, 4), (36, 4), (40, 2)]

ROWC = UP * SB  # bridge cols per row: (u 168, s 256)


def build_program():
    nc = bacc.Bacc("TRN2", target_bir_lowering=False)
    xk2 = nc.declare_dram_parameter("xk2", [V, NG, F1 * SB], BF, False)
    brs1 = nc.declare_dram_parameter("brs1", [NG, 80, ROWC], BF, False)
    tsk = nc.declare_dram_parameter("tsk", [V, 648], BF, False)
    w1r = nc.declare_dram_parameter("w1r", [K * F1, F2], BF, False)
    w2a = nc.declare_dram_parameter("w2a", [128, F2], BF, False)
    w20b = nc.declare_dram_parameter("w20b", [128, 128], BF, False)
    gb1 = nc.declare_dram_parameter("gb1", [128, 2], F32, False)
    gb2 = nc.declare_dram_parameter("gb2", [128, 2], F32, False)
    wrow = nc.declare_dram_parameter("wrow", [128, 1], F32, False)
    out = nc.declare_dram_parameter("out", [F2, V, S], BF, isOutput=True)

    with tile.TileContext(nc) as tc:
        with (
            tc.tile_pool(name="consts", bufs=1) as cpool,
            tc.tile_pool(name="slab", bufs=1) as slab,
            tc.tile_pool(name="stats", bufs=1) as spool,
            tc.tile_pool(name="dram", bufs=2, space="DRAM") as dram,
        ):
            tA = cpool.tile([VA, 648], BF)
            tB = cpool.tile([VB, 648], BF)
            w1t = cpool.tile([K * F1, F2], BF)
            w2at = cpool.tile([128, F2], BF)
            w20t = cpool.tile([128, 128], BF)
            gb1t = cpool.tile([128, 2], F32)
            gb2t = cpool.tile([128, 2], F32)
            wrt = cpool.tile([128, 1], F32)
            nc.sync.dma_start(tA[:], tsk[0:VA, :])
            nc.sync.dma_start(tB[:], tsk[VA:V, :])
            nc.sync.dma_start(w1t[:], w1r[:])
            nc.sync.dma_start(w2at[:], w2a[:])
            nc.sync.dma_start(w20t[:], w20b[:])
            nc.sync.dma_start(gb1t[:], gb1[:])
            nc.sync.dma_start(gb2t[:], gb2[:])
            nc.sync.dma_start(wrt[:], wrow[:])

            yslab = slab.tile([128, 42 * S], BF)
            ygr = yslab[:, :].rearrange("p (u s) -> p u s", u=42, s=S)
            nc.gpsimd.memset(ygr[96:128, 36:42, :], 0.0)
            par1 = spool.tile([128, 2], F32)
            par2 = spool.tile([128, 2], F32)

            def bn_finalize(stscr, gbt, par, tag):
                sv = stscr[:, :].rearrange("p (n e) -> p n e", n=NSLOT, e=8)
                mv = spool.tile([128, 2], F32, tag=f"mv{tag}")
                nc.vector.bn_aggr(mv[:], sv[:, :, 0:6])
                es = spool.tile([128, 2], F32, tag=f"es{tag}")
                nc.vector.tensor_mul(es[:, 1:2], mv[:, 0:1], mv[:, 0:1])
                nc.vector.tensor_add(es[:, 1:2], es[:, 1:2], mv[:, 1:2])
                nc.vector.tensor_copy(es[:, 0:1], mv[:, 0:1])
                nc.vector.tensor_mul(es[:, 0:1], es[:, 0:1], wrt[:, 0:1])
                nc.vector.tensor_mul(es[:, 1:2], es[:, 1:2], wrt[:, 0:1])
                # quadrant pre-fold through DRAM, then tiny AllReduce
                esd = dram.tile([128, 2], F32, tag=f"esd{tag}")
                nc.sync.dma_start(esd[:], es[:])
                qs = spool.tile([32, 8], F32, tag=f"qs{tag}")
                nc.sync.dma_start(
                    qs[:].rearrange("o (j e) -> o j e", j=4, e=2),
                    esd[:].rearrange("(j o) e -> o j e", j=4, o=32))
                fold = spool.tile([32, 4], F32, tag=f"fold{tag}")
                nc.vector.tensor_add(fold[:, 0:2], qs[:, 0:2], qs[:, 2:4])
                nc.vector.tensor_add(fold[:, 2:4], qs[:, 4:6], qs[:, 6:8])
                nc.vector.tensor_add(fold[:, 0:2], fold[:, 0:2],
                                     fold[:, 2:4])
                cin = dram.tile([32, 2], F32, tag=f"cin{tag}")
                cout = dram.tile([32, 2], F32, tag=f"cout{tag}")
                nc.gpsimd.dma_start(cin[:], fold[:, 0:2])
                if not SKIP_CC:
                    nc.gpsimd.collective_compute(
                        "AllReduce", mybir.AluOpType.add,
                        replica_groups=[list(range(N_CORES))],
                        ins=[cin[:].opt()], outs=[cout[:].opt()])
                else:
                    nc.gpsimd.dma_start(cout[:], cin[:])
                acc = spool.tile([32, 6], F32, tag=f"acc{tag}")
                nc.sync.dma_start(acc[:, 0:2], cout[:])
                nc.vector.tensor_mul(acc[:, 2:3], acc[:, 0:1], acc[:, 0:1])
                nc.vector.tensor_sub(acc[:, 1:2], acc[:, 1:2], acc[:, 2:3])
                nc.vector.tensor_scalar_add(acc[:, 1:2], acc[:, 1:2], EPS)
                nc.scalar.sqrt(acc[:, 2:3], acc[:, 1:2])
                nc.vector.reciprocal(acc[:, 3:4], acc[:, 2:3])
                nc.vector.tensor_mul(acc[:, 4:5], gbt[0:32, 0:1], acc[:, 3:4])
                nc.vector.tensor_mul(acc[:, 5:6], acc[:, 0:1], acc[:, 4:5])
                nc.vector.tensor_sub(acc[:, 5:6], gbt[0:32, 1:2], acc[:, 5:6])
                for j in range(4):
                    nc.sync.dma_start(par[32 * j:32 * j + 32, 0:2],
                                      acc[:, 4:6])

            def group_cheb_bridge(b, hha, hhb, xmp, brw, brv, G, m1ps, tagp):
                # m-major: both chunks' cheb for m-tile, then its span DMAs
                # hha/hhb: [v, (g, s2, s)] 2-chunk rhs tiles
                nq = G * SC // 512
                hav = hha[:, :].rearrange("p (g t s) -> p g t s",
                                          g=G, t=2, s=SC)
                hbv = hhb[:, :].rearrange("p (g t s) -> p g t s",
                                          g=G, t=2, s=SC)
                for m in range(6):
                    xm = xmp.tile([108, G * SB], BF, tag="xm",
                                  name=f"xm_{b}_{m}")
                    xmv = xm[:, :].rearrange("p (g s) -> p g s", g=G, s=SB)
                    for cc in range(2):
                        pss = [m1ps.tile([108, 512], F32, tag=tagp,
                                         name=f"{tagp}_{b}_{m}_{cc}_{q}")
                               for q in range(nq)]
                        for tt, xx in ((tA, hav), (tB, hbv)):
                            lw = tt[:, m * 108:(m + 1) * 108]
                            for q in range(nq):
                                nc.tensor.matmul(
                                    pss[q][:], lw,
                                    xx[:, 4 * q:4 * q + 4, cc, :],
                                    start=(tt is tA), stop=(tt is tB))
                        for q in range(nq):
                            src = pss[q][:, :].rearrange(
                                "p (g s) -> p g s", g=4, s=SC)
                            dst = xmv[:, 4 * q:4 * q + 4,
                                      cc * SC:(cc + 1) * SC]
                            if q % 4 == 3:
                                nc.scalar.copy(dst, src)
                            else:
                                nc.vector.tensor_copy(dst, src)
                    for i, (k, t, r0, u0, du) in enumerate(
                            SPANS_BY_T.get(m, [])):
                        src = xm[r0:r0 + du, :].rearrange(
                            "u (g s) -> u g s", g=G, s=SB)
                        dst = brv[G * (k - 1) + brw:G * k + brw,
                                  u0:u0 + du, :] \
                            .rearrange("g u s -> u g s")
                        eng = nc.sync if (m + i) % 2 else nc.scalar
                        eng.dma_start(dst, src)

            def proj_slot(c, si, sv, lhsT, m2ps, stscr, l2):
                sl = c * SC
                half = (c % 2) * SC
                u0, du = USLOTS[si]
                rows = 128 if u0 + du <= 36 else 96
                nw = du * SC
                ps2 = m2ps.tile([128, 512], F32, tag=f"m2ps{l2}")
                rows0 = 128 if u0 + du <= 36 else 96
                for j in range(4):
                    if u0 >= 36 and j == 3:
                        continue
                    rhs = sv[:, j, 0:du, half:half + SC]
                    nc.tensor.matmul(
                        ps2[32 * j:32 * j + 32, 0:nw], lhsT, rhs,
                        start=True, stop=not l2,
                        tile_position=(0, 32 * j))
                if l2:
                    # fused k0 term: block-diag kron(I4, w20) over all bands
                    nc.tensor.matmul(
                        ps2[0:rows0, 0:nw], w20t[:, 0:rows0],
                        ygr[:, u0:u0 + du, sl:sl + SC],
                        start=False, stop=True)
                slot = c * 14 + si
                st = stscr[0:rows, slot * 8:slot * 8 + 6]
                nc.vector.bn_stats(st, ps2[0:rows, 0:nw])
                dst = ygr[0:rows, u0:u0 + du, sl:sl + SC]
                src = ps2[0:rows, 0:nw].rearrange(
                    "p (r s) -> p r s", r=du, s=SC)
                if si % 2 == 0:
                    nc.scalar.copy(dst, src)
                else:
                    nc.vector.tensor_copy(dst, src)

            def slot_reads(brS, rows, slp, tag):
                brju = brS[:, :].rearrange("p (j us) -> p j us",
                                           j=4, us=42 * SB)
                slots = []
                for si in range(11):
                    rw = 4 if si < 10 else 2
                    st = slp.tile([rows, 16 * SB], BF, tag=tag)
                    stv = st[:, 0:4 * rw * SB].rearrange(
                        "p (j r s) -> p j r s", j=4, r=rw, s=SB)
                    st3 = st[:, 0:4 * rw * SB].rearrange(
                        "p (j rs) -> p j rs", j=4, rs=rw * SB)
                    eng = nc.sync if si % 2 else nc.scalar
                    eng.dma_start(
                        st3[:],
                        brju[:, :, 4 * si * SB:(4 * si + rw) * SB])
                    slots.append(stv)
                return slots

            # ---------------- layer 1 ----------------
            with (
                tc.tile_pool(name="x", bufs=3) as xpool,
                tc.tile_pool(name="xb", bufs=3) as xbp,
                tc.tile_pool(name="xm", bufs=4) as xmp,
                tc.tile_pool(name="sl1", bufs=4) as slp1,
                tc.tile_pool(name="m1ps", bufs=5, space="PSUM") as m1ps,
                tc.tile_pool(name="m2ps", bufs=3, space="PSUM") as m2ps,
                tc.tile_pool(name="s1", bufs=1) as s1pool,
            ):
                stscr1 = s1pool.tile([128, STSCR_W], F32)
                nc.gpsimd.memset(stscr1[:], 0.0)

                def l1_front(b):
                    xa = xpool.tile([VA, F1 * SB], BF, tag="xa")
                    xb = xbp.tile([VB, F1 * SB], BF, tag="xb")
                    nc.sync.dma_start(xa[:], xk2[0:VA, b, :])
                    nc.scalar.dma_start(xb[:], xk2[VA:V, b, :])
                    brS = brs1[b]
                    brv = brS[:, :].rearrange("p (u s) -> p u s",
                                              u=UP, s=SB)
                    group_cheb_bridge(b, xa, xb, xmp, 16, brv, F1,
                                      m1ps, "m1ps")
                    return slot_reads(brS, 80, slp1, "sl1")

                def l1_proj(b, slots):
                    for si in range(11):
                        for cc in range(2):
                            proj_slot(2 * b + cc, si, slots[si],
                                      w1t[:], m2ps, stscr1, False)

                pend = {}
                for b in range(NG):
                    pend[b] = l1_front(b)
                    if b >= 1:
                        l1_proj(b - 1, pend.pop(b - 1))
                l1_proj(NG - 1, pend.pop(NG - 1))
                bn_finalize(stscr1, gb1t, par1, "1")

            # ---------------- layer 2 ----------------
            with (
                tc.tile_pool(name="ha", bufs=2) as hap,
                tc.tile_pool(name="hb", bufs=1) as hbp,
                tc.tile_pool(name="xm2", bufs=2) as xmp2,
                tc.tile_pool(name="sl2", bufs=4) as slp2,
                tc.tile_pool(name="m1ps2", bufs=5, space="PSUM") as m1ps2,
                tc.tile_pool(name="m2ps2", bufs=3, space="PSUM") as m2ps2,
                tc.tile_pool(name="s2", bufs=1) as s2pool,
            ):
                stscr2 = s2pool.tile([128, STSCR_W], F32)
                nc.gpsimd.memset(stscr2[:], 0.0)

                def l2_front(b):
                    sl2 = b * SB
                    for cc in range(2):
                        sl = sl2 + cc * SC
                        nc.scalar.activation(
                            ygr[:, :, sl:sl + SC], ygr[:, :, sl:sl + SC],
                            mybir.ActivationFunctionType.Relu,
                            bias=par1[:, 1:2], scale=par1[:, 0:1])
                    bh = dram.tile([128, 42 * SB], BF, tag="bh")
                    nc.sync.dma_start(
                        bh[:, :].rearrange("p (u s) -> p u s", u=42, s=SB),
                        ygr[:, :, sl2:sl2 + SB])
                    bhv = bh[:, :].rearrange(
                        "(j o) (u s) -> j o u s", j=4, o=F2, u=42, s=SB)
                    ha = hap.tile([VA, F2 * SB], BF, tag="ha")
                    hb = hbp.tile([VB, F2 * SB], BF, tag="hb")
                    hav = ha[:, :].rearrange("p (o s) -> p o s", o=F2, s=SB)
                    hbv = hb[:, :].rearrange("p (o s) -> p o s", o=F2, s=SB)
                    for j in range(3):
                        eng = nc.sync if j % 2 else nc.scalar
                        eng.dma_start(
                            hav[42 * j:42 * j + 42],
                            bhv[j].rearrange("o u s -> u o s"))
                    nc.scalar.dma_start(
                        hbv[:], bhv[3, :, 0:VB, :].rearrange("o u s -> u o s"))
                    brS = dram.tile([128, ROWC], BF, tag="brS2")
                    brv = brS[:, :].rearrange("p (u s) -> p u s",
                                              u=UP, s=SB)
                    group_cheb_bridge(b, ha, hb, xmp2, 0, brv, F2,
                                      m1ps2, "m1ps2")
                    return slot_reads(brS, 128, slp2, "sl2")

                def l2_proj(b, slots):
                    for si in range(11):
                        for cc in range(2):
                            proj_slot(2 * b + cc, si, slots[si],
                                      w2at[:], m2ps2, stscr2, True)

                pend = {}
                for b in range(NG):
                    pend[b] = l2_front(b)
                    if b >= 1:
                        l2_proj(b - 1, pend.pop(b - 1))
                l2_proj(NG - 1, pend.pop(NG - 1))
                bn_finalize(stscr2, gb2t, par2, "2")

            # ---- final normalize + relu + store (s-chunked, per band) ----
            with tc.tile_pool(name="stg", bufs=2) as stg:
                SP = 256
                for p in range(S // SP):
                    so = stg.tile([128, 42 * SP], BF, tag="so")
                    sov = so[:, :].rearrange("p (u s) -> p u s", u=42, s=SP)
                    nc.scalar.activation(
                        sov[:], ygr[:, :, p * SP:(p + 1) * SP],
                        mybir.ActivationFunctionType.Relu,
                        bias=par2[:, 1:2], scale=par2[:, 0:1])
                    for bb in range(4):
                        r0, w = 32 * bb, BW[bb]
                        nc.sync.dma_start(
                            out[:, UB[bb]:UB[bb + 1], p * SP:(p + 1) * SP],
                            sov[r0:r0 + 32, 0:w, :])
    nc.compile()
    return nc


def _host_prep(x, lap, w1, w2, g1, be1, g2, be2):
    lap64 = np.asarray(lap).astype(np.float64)
    T = [np.eye(V), lap64]
    for _ in range(2, K):
        T.append(2.0 * lap64 @ T[-1] - T[-2])
    tsk = np.concatenate([T[k].T for k in range(1, K)], axis=1)  # [162, 648]
    w1f = np.asarray(w1).reshape(K * F1, F2)
    w2f = np.asarray(w2).reshape(K * F2, F2)
    gb1 = np.stack([np.tile(np.asarray(g1), 4), np.tile(np.asarray(be1), 4)],
                   axis=1)
    gb2 = np.stack([np.tile(np.asarray(g2), 4), np.tile(np.asarray(be2), 4)],
                   axis=1)
    nrow = np.repeat(np.array(BW, np.float64) * S, 32)
    denom = (1.0 if SKIP_CC else float(N_CORES)) * V * S
    wrow = (nrow / denom).astype(np.float32)[:, None]
    common = {
        "tsk": tsk.astype(BF16),
        "w1r": w1f.astype(BF16),
        "w2a": w2f[F2:].astype(BF16),
        "w20b": np.kron(np.eye(4), w2f[0:F2]).astype(BF16),
        "gb1": gb1.astype(np.float32), "gb2": gb2.astype(np.float32),
        "wrow": wrow,
    }
    in_maps = []
    xf = np.asarray(x).reshape(2, F1, V, 4096)
    for core in range(N_CORES):
        b, q = core // 4, core % 4
        xs = xf[b, :, :, q * S:(q + 1) * S]            # [16, 162, 1024]
        # xk2: [V, NG, F1*SB]
        xkc = xs.transpose(1, 0, 2).reshape(V, F1, NG, SB)
        xkc = xkc.transpose(0, 2, 1, 3).reshape(V, NG, F1 * SB)
        # brs1: [NG, 80, (u 168, s 256)]; rows 0:16 = k0 = x itself
        br = np.zeros((NG, 80, UP, SB), np.float32)
        br[:, 0:F1, 0:V, :] = xs.reshape(F1, V, NG, SB).transpose(2, 0, 1, 3)
        m = dict(common)
        m["xk2"] = np.ascontiguousarray(xkc).astype(BF16)
        m["brs1"] = np.ascontiguousarray(
            br.reshape(NG, 80, ROWC)).astype(BF16)
        in_maps.append(m)
    return in_maps


_CACHE = {}


def _run(in_maps, trace=False):
    if "nc" not in _CACHE:
        _CACHE["nc"] = build_program()
    return run_bass_kernel_spmd(
        _CACHE["nc"], in_maps, core_ids=list(range(N_CORES)), trace=trace)


def kernel(x, lap, w1, b1, g1, be1, w2, b2, g2, be2, _trace=False):
    # conv biases b1/b2 cancel exactly inside BatchNorm -> ignored
    in_maps = _host_prep(x, lap, w1, w2, g1, be1, g2, be2)
    res = _run(in_maps, trace=_trace)
    _CACHE["last_results"] = res
    full = np.empty((2, F2, V, 4096), np.float32)
    for core in range(N_CORES):
        b, q = core // 4, core % 4
        full[b, :, :, q * S:(q + 1) * S] = \
            res.results[core]["out"].astype(np.float32)
    return full.reshape(2, F2, V, 16, 16, 16)


# revision 12
# speedup vs baseline: 1.1649x; 1.1649x over previous
"""Trainium2 Bass kernel v5 for nn_Block_39513699123558 (gnn_message_passing).

Two layers of (Chebyshev graph conv K=5 -> BatchNorm -> ReLU) on
x[B=2, F0=16, V=162, X=Y=Z=16].

v5: vertex-major bridge DRAM layout at 2-chunk (s=256) granularity.
  brS rows = (k, ch-out), cols = (padded vertex 168, s 256).  leg1 is
  ONE scatter DMA per T-stack span (512B runs); leg2 is one contiguous
  2KB-run DMA per projection u-slot into a tiny [*, 8KB] slot tile.
  Layer-1 k=0 rows are pre-staged by the host inside the bridge
  parameter; layer-2 k=0 reads the activated y-slab in place.
  Inputs and the h-bridge load at 2-chunk granularity (512B runs).
  Cheb contraction split 126+36 so band 3 is one clean piece.
"""

import os
import sys

sys.path.insert(0, "/opt/trn_rl_repo")

SKIP_CC = os.environ.get("K_SKIP_CC", "0") == "1"

import numpy as np
import ml_dtypes

from concourse import bass, bacc, mybir
from concourse import tile
from concourse.bass_utils import run_bass_kernel_spmd

BF16 = ml_dtypes.bfloat16
BF = mybir.dt.bfloat16
F32 = mybir.dt.float32

V = 162
VA = 126
VB = V - VA  # 36
F1, F2 = 16, 32
K = 5
S = 1024          # s-columns per core
SC = 128          # compute s-chunk
SB = 256          # bridge s-granularity (2 chunks)
NG = S // SB      # 4 groups
UP = 168          # padded vertex cols (4 bands x 42)
EPS = 1e-5
N_CORES = 8

UB = [0, 42, 84, 126, 162]
BW = [42, 42, 42, 36]

# T-stack k=1..4, 648 rows, 6 tiles of 108; spans (k, t, r0, u0, du)
SP1 = [(1, 0, 0, 0, 108), (1, 1, 0, 108, 54),
       (2, 1, 54, 0, 54), (2, 2, 0, 54, 108),
       (3, 3, 0, 0, 108), (3, 4, 0, 108, 54),
       (4, 4, 54, 0, 54), (4, 5, 0, 54, 108)]
SPANS_BY_T = {}
for _sp in SP1:
    SPANS_BY_T.setdefault(_sp[1], []).append(_sp)

NSLOT = 112
STSCR_W = NSLOT * 8
USLOTS = [(0, 4), (4, 4), (8, 4), (12, 4), (16, 4), (20, 4),
          (24, 4), (28, 4), (32, 4), (36, 4), (40, 2)]

ROWC = UP * SB  # bridge cols per row: (u 168, s 256)


def build_program():
    nc = bacc.Bacc("TRN2", target_bir_lowering=False)
    xk2 = nc.declare_dram_parameter("xk2", [V, NG, F1 * SB], BF, False)
    brs1 = nc.declare_dram_parameter("brs1", [NG, 80, ROWC], BF, False)
    tsk = nc.declare_dram_parameter("tsk", [V, 648], BF, False)
    w1r = nc.declare_dram_parameter("w1r", [K * F1, F2], BF, False)
    w2a = nc.declare_dram_parameter("w2a", [128, F2], BF, False)
    w20b = nc.declare_dram_parameter("w20b", [128, 128], BF, False)
    gb1 = nc.declare_dram_parameter("gb1", [128, 2], F32, False)
    gb2 = nc.declare_dram_parameter("gb2", [128, 2], F32, False)
    wrow = nc.declare_dram_parameter("wrow", [128, 1], F32, False)
    out = nc.declare_dram_parameter("out", [F2, V, S], BF, isOutput=True)

    with tile.TileContext(nc) as tc:
        with (
            tc.tile_pool(name="consts", bufs=1) as cpool,
            tc.tile_pool(name="slab", bufs=1) as slab,
            tc.tile_pool(name="stats", bufs=1) as spool,
            tc.tile_pool(name="dram", bufs=2, space="DRAM") as dram,
        ):
            tA = cpool.tile([VA, 648], BF)
            tB = cpool.tile([VB, 648], BF)
            w1t = cpool.tile([K * F1, F2], BF)
            w2at = cpool.tile([128, F2], BF)
            w20t = cpool.tile([128, 128], BF)
            gb1t = cpool.tile([128, 2], F32)
            gb2t = cpool.tile([128, 2], F32)
            wrt = cpool.tile([128, 1], F32)
            nc.sync.dma_start(tA[:], tsk[0:VA, :])
            nc.sync.dma_start(tB[:], tsk[VA:V, :])
            nc.sync.dma_start(w1t[:], w1r[:])
            nc.sync.dma_start(w2at[:], w2a[:])
            nc.sync.dma_start(w20t[:], w20b[:])
            nc.sync.dma_start(gb1t[:], gb1[:])
            nc.sync.dma_start(gb2t[:], gb2[:])
            nc.sync.dma_start(wrt[:], wrow[:])

            yslab = slab.tile([128, 42 * S], BF)
            ygr = yslab[:, :].rearrange("p (u s) -> p u s", u=42, s=S)
            nc.gpsimd.memset(ygr[96:128, 36:42, :], 0.0)
            par1 = spool.tile([128, 2], F32)
            par2 = spool.tile([128, 2], F32)

            def bn_finalize(stscr, gbt, par, tag):
                sv = stscr[:, :].rearrange("p (n e) -> p n e", n=NSLOT, e=8)
                mv = spool.tile([128, 2], F32, tag=f"mv{tag}")
                nc.vector.bn_aggr(mv[:], sv[:, :, 0:6])
                es = spool.tile([128, 2], F32, tag=f"es{tag}")
                nc.vector.tensor_mul(es[:, 1:2], mv[:, 0:1], mv[:, 0:1])
                nc.vector.tensor_add(es[:, 1:2], es[:, 1:2], mv[:, 1:2])
                nc.vector.tensor_copy(es[:, 0:1], mv[:, 0:1])
                nc.vector.tensor_mul(es[:, 0:1], es[:, 0:1], wrt[:, 0:1])
                nc.vector.tensor_mul(es[:, 1:2], es[:, 1:2], wrt[:, 0:1])
                # quadrant pre-fold through DRAM, then tiny AllReduce
                esd = dram.tile([128, 2], F32, tag=f"esd{tag}")
                nc.sync.dma_start(esd[:], es[:])
                qs = spool.tile([32, 8], F32, tag=f"qs{tag}")
                nc.sync.dma_start(
                    qs[:].rearrange("o (j e) -> o j e", j=4, e=2),
                    esd[:].rearrange("(j o) e -> o j e", j=4, o=32))
                fold = spool.tile([32, 4], F32, tag=f"fold{tag}")
                nc.vector.tensor_add(fold[:, 0:2], qs[:, 0:2], qs[:, 2:4])
                nc.vector.tensor_add(fold[:, 2:4], qs[:, 4:6], qs[:, 6:8])
                nc.vector.tensor_add(fold[:, 0:2], fold[:, 0:2],
                                     fold[:, 2:4])
                cin = dram.tile([32, 2], F32, tag=f"cin{tag}")
                cout = dram.tile([32, 2], F32, tag=f"cout{tag}")
                nc.gpsimd.dma_start(cin[:], fold[:, 0:2])
                if not SKIP_CC:
                    nc.gpsimd.collective_compute(
                        "AllReduce", mybir.AluOpType.add,
                        replica_groups=[list(range(N_CORES))],
                        ins=[cin[:].opt()], outs=[cout[:].opt()])
                else:
                    nc.gpsimd.dma_start(cout[:], cin[:])
                acc = spool.tile([32, 6], F32, tag=f"acc{tag}")
                nc.sync.dma_start(acc[:, 0:2], cout[:])
                nc.vector.tensor_mul(acc[:, 2:3], acc[:, 0:1], acc[:, 0:1])
                nc.vector.tensor_sub(acc[:, 1:2], acc[:, 1:2], acc[:, 2:3])
                nc.vector.tensor_scalar_add(acc[:, 1:2], acc[:, 1:2], EPS)
                nc.scalar.sqrt(acc[:, 2:3], acc[:, 1:2])
                nc.vector.reciprocal(acc[:, 3:4], acc[:, 2:3])
                nc.vector.tensor_mul(acc[:, 4:5], gbt[0:32, 0:1], acc[:, 3:4])
                nc.vector.tensor_mul(acc[:, 5:6], acc[:, 0:1], acc[:, 4:5])
                nc.vector.tensor_sub(acc[:, 5:6], gbt[0:32, 1:2], acc[:, 5:6])
                for j in range(4):
                    nc.sync.dma_start(par[32 * j:32 * j + 32, 0:2],
                                      acc[:, 4:6])

            def group_cheb_bridge(b, hha, hhb, xmp, brw, brv, G, m1ps, tagp):
                # m-major: both chunks' cheb for m-tile, then its span DMAs
                # hha/hhb: [v, (g, s2, s)] 2-chunk rhs tiles
                nq = G * SC // 512
                hav = hha[:, :].rearrange("p (g t s) -> p g t s",
                                          g=G, t=2, s=SC)
                hbv = hhb[:, :].rearrange("p (g t s) -> p g t s",
                                          g=G, t=2, s=SC)
                for m in range(6):
                    xm = xmp.tile([108, G * SB], BF, tag="xm",
                                  name=f"xm_{b}_{m}")
                    xmv = xm[:, :].rearrange("p (g s) -> p g s", g=G, s=SB)
                    for cc in range(2):
                        pss = [m1ps.tile([108, 512], F32, tag=tagp,
                                         name=f"{tagp}_{b}_{m}_{cc}_{q}")
                               for q in range(nq)]
                        for tt, xx in ((tA, hav), (tB, hbv)):
                            lw = tt[:, m * 108:(m + 1) * 108]
                            for q in range(nq):
                                nc.tensor.matmul(
                                    pss[q][:], lw,
                                    xx[:, 4 * q:4 * q + 4, cc, :],
                                    start=(tt is tA), stop=(tt is tB))
                        for q in range(nq):
                            src = pss[q][:, :].rearrange(
                                "p (g s) -> p g s", g=4, s=SC)
                            dst = xmv[:, 4 * q:4 * q + 4,
                                      cc * SC:(cc + 1) * SC]
                            if q % 4 == 3:
                                nc.scalar.copy(dst, src)
                            else:
                                nc.vector.tensor_copy(dst, src)
                    for i, (k, t, r0, u0, du) in enumerate(
                            SPANS_BY_T.get(m, [])):
                        src = xm[r0:r0 + du, :].rearrange(
                            "u (g s) -> u g s", g=G, s=SB)
                        dst = brv[G * (k - 1) + brw:G * k + brw,
                                  u0:u0 + du, :] \
                            .rearrange("g u s -> u g s")
                        eng = nc.sync if (m + i) % 2 else nc.scalar
                        eng.dma_start(dst, src)

            def proj_slot(c, si, sv, lhsT, m2ps, stscr, l2):
                sl = c * SC
                half = (c % 2) * SC
                u0, du = USLOTS[si]
                rows = 128 if u0 + du <= 36 else 96
                nw = du * SC
                ps2 = m2ps.tile([128, 512], F32, tag=f"m2ps{l2}")
                rows0 = 128 if u0 + du <= 36 else 96
                for j in range(4):
                    if u0 >= 36 and j == 3:
                        continue
                    rhs = sv[:, j, 0:du, half:half + SC]
                    nc.tensor.matmul(
                        ps2[32 * j:32 * j + 32, 0:nw], lhsT, rhs,
                        start=True, stop=not l2,
                        tile_position=(0, 32 * j))
                if l2:
                    # fused k0 term: block-diag kron(I4, w20) over all bands
                    nc.tensor.matmul(
                        ps2[0:rows0, 0:nw], w20t[:, 0:rows0],
                        ygr[:, u0:u0 + du, sl:sl + SC],
                        start=False, stop=True)
                slot = c * 14 + si
                st = stscr[0:rows, slot * 8:slot * 8 + 6]
                nc.vector.bn_stats(st, ps2[0:rows, 0:nw])
                dst = ygr[0:rows, u0:u0 + du, sl:sl + SC]
                src = ps2[0:rows, 0:nw].rearrange(
                    "p (r s) -> p r s", r=du, s=SC)
                if si % 2 == 0:
                    nc.scalar.copy(dst, src)
                else:
                    nc.vector.tensor_copy(dst, src)

            def slot_reads(brS, rows, slp, tag):
                brju = brS[:, :].rearrange("p (j us) -> p j us",
                                           j=4, us=42 * SB)
                slots = []
                for si in range(11):
                    rw = 4 if si < 10 else 2
                    st = slp.tile([rows, 16 * SB], BF, tag=tag)
                    stv = st[:, 0:4 * rw * SB].rearrange(
                        "p (j r s) -> p j r s", j=4, r=rw, s=SB)
                    st3 = st[:, 0:4 * rw * SB].rearrange(
                        "p (j rs) -> p j rs", j=4, rs=rw * SB)
                    eng = nc.sync if si % 2 else nc.scalar
                    eng.dma_start(
                        st3[:],
                        brju[:, :, 4 * si * SB:(4 * si + rw) * SB])
                    slots.append(stv)
                return slots

            # ---------------- layer 1 ----------------
            with (
                tc.tile_pool(name="x", bufs=2) as xpool,
                tc.tile_pool(name="xb", bufs=2) as xbp,
                tc.tile_pool(name="xm", bufs=3) as xmp,
                tc.tile_pool(name="sl1", bufs=4) as slp1,
                tc.tile_pool(name="m1ps", bufs=5, space="PSUM") as m1ps,
                tc.tile_pool(name="m2ps", bufs=3, space="PSUM") as m2ps,
                tc.tile_pool(name="s1", bufs=1) as s1pool,
            ):
                stscr1 = s1pool.tile([128, STSCR_W], F32)
                nc.gpsimd.memset(stscr1[:], 0.0)

                def l1_front(b):
                    xa = xpool.tile([VA, F1 * SB], BF, tag="xa")
                    xb = xbp.tile([VB, F1 * SB], BF, tag="xb")
                    nc.sync.dma_start(xa[:], xk2[0:VA, b, :])
                    nc.scalar.dma_start(xb[:], xk2[VA:V, b, :])
                    brS = brs1[b]
                    brv = brS[:, :].rearrange("p (u s) -> p u s",
                                              u=UP, s=SB)
                    group_cheb_bridge(b, xa, xb, xmp, 16, brv, F1,
                                      m1ps, "m1ps")
                    return slot_reads(brS, 80, slp1, "sl1")

                def l1_proj(b, slots):
                    for si in range(11):
                        for cc in range(2):
                            proj_slot(2 * b + cc, si, slots[si],
                                      w1t[:], m2ps, stscr1, False)

                pend = {}
                for b in range(NG):
                    pend[b] = l1_front(b)
                    if b >= 1:
                        l1_proj(b - 1, pend.pop(b - 1))
                l1_proj(NG - 1, pend.pop(NG - 1))
                bn_finalize(stscr1, gb1t, par1, "1")

            # ---------------- layer 2 ----------------
            with (
                tc.tile_pool(name="ha", bufs=2) as hap,
                tc.tile_pool(name="hb", bufs=1) as hbp,
                tc.tile_pool(name="xm2", bufs=2) as xmp2,
                tc.tile_pool(name="sl2", bufs=4) as slp2,
                tc.tile_pool(name="m1ps2", bufs=5, space="PSUM") as m1ps2,
                tc.tile_pool(name="m2ps2", bufs=3, space="PSUM") as m2ps2,
                tc.tile_pool(name="s2", bufs=1) as s2pool,
            ):
                stscr2 = s2pool.tile([128, STSCR_W], F32)
                nc.gpsimd.memset(stscr2[:], 0.0)

                def l2_front(b):
                    sl2 = b * SB
                    for cc in range(2):
                        sl = sl2 + cc * SC
                        nc.scalar.activation(
                            ygr[:, :, sl:sl + SC], ygr[:, :, sl:sl + SC],
                            mybir.ActivationFunctionType.Relu,
                            bias=par1[:, 1:2], scale=par1[:, 0:1])
                    bh = dram.tile([128, 42 * SB], BF, tag="bh")
                    nc.sync.dma_start(
                        bh[:, :].rearrange("p (u s) -> p u s", u=42, s=SB),
                        ygr[:, :, sl2:sl2 + SB])
                    bhv = bh[:, :].rearrange(
                        "(j o) (u s) -> j o u s", j=4, o=F2, u=42, s=SB)
                    ha = hap.tile([VA, F2 * SB], BF, tag="ha")
                    hb = hbp.tile([VB, F2 * SB], BF, tag="hb")
                    hav = ha[:, :].rearrange("p (o s) -> p o s", o=F2, s=SB)
                    hbv = hb[:, :].rearrange("p (o s) -> p o s", o=F2, s=SB)
                    for j in range(3):
                        eng = nc.sync if j % 2 else nc.scalar
                        eng.dma_start(
                            hav[42 * j:42 * j + 42],
                            bhv[j].rearrange("o u s -> u o s"))
                    nc.scalar.dma_start(
                        hbv[:], bhv[3, :, 0:VB, :].rearrange("o u s -> u o s"))
                    brS = dram.tile([128, ROWC], BF, tag="brS2")
                    brv = brS[:, :].rearrange("p (u s) -> p u s",
                                              u=UP, s=SB)
                    group_cheb_bridge(b, ha, hb, xmp2, 0, brv, F2,
                                      m1ps2, "m1ps2")
                    return slot_reads(brS, 128, slp2, "sl2")

                def l2_proj(b, slots):
                    for si in range(11):
                        for cc in range(2):
                            proj_slot(2 * b + cc, si, slots[si],
                                      w2at[:], m2ps2, stscr2, True)

                pend = {}
                for b in range(NG):
                    pend[b] = l2_front(b)
                    if b >= 1:
                        l2_proj(b - 1, pend.pop(b - 1))
                l2_proj(NG - 1, pend.pop(NG - 1))
                bn_finalize(stscr2, gb2t, par2, "2")

            # ---- final normalize + relu + store (s-chunked, per band) ----
            with tc.tile_pool(name="stg", bufs=2) as stg:
                SP = 256
                for p in range(S // SP):
                    so = stg.tile([128, 42 * SP], BF, tag="so")
                    sov = so[:, :].rearrange("p (u s) -> p u s", u=42, s=SP)
                    nc.scalar.activation(
                        sov[:], ygr[:, :, p * SP:(p + 1) * SP],
                        mybir.ActivationFunctionType.Relu,
                        bias=par2[:, 1:2], scale=par2[:, 0:1])
                    for bb in range(4):
                        r0, w = 32 * bb, BW[bb]
                        eng = nc.sync if bb % 2 else nc.scalar
                        eng.dma_start(
                            out[:, UB[bb]:UB[bb + 1], p * SP:(p + 1) * SP],
                            sov[r0:r0 + 32, 0:w, :])
    nc.compile()
    return nc


def _host_prep(x, lap, w1, w2, g1, be1, g2, be2):
    lap64 = np.asarray(lap).astype(np.float64)
    T = [np.eye(V), lap64]
    for _ in range(2, K):
        T.append(2.0 * lap64 @ T[-1] - T[-2])
    tsk = np.concatenate([T[k].T for k in range(1, K)], axis=1)  # [162, 648]
    w1f = np.asarray(w1).reshape(K * F1, F2)
    w2f = np.asarray(w2).reshape(K * F2, F2)
    gb1 = np.stack([np.tile(np.asarray(g1), 4), np.tile(np.asarray(be1), 4)],
                   axis=1)
    gb2 = np.stack([np.tile(np.asarray(g2), 4), np.tile(np.asarray(be2), 4)],
                   axis=1)
    nrow = np.repeat(np.array(BW, np.float64) * S, 32)
    denom = (1.0 if SKIP_CC else float(N_CORES)) * V * S
    wrow = (nrow / denom).astype(np.float32)[:, None]
    common = {
        "tsk": tsk.astype(BF16),
        "w1r": w1f.astype(BF16),
        "w2a": w2f[F2:].astype(BF16),
        "w20b": np.kron(np.eye(4), w2f[0:F2]).astype(BF16),
        "gb1": gb1.astype(np.float32), "gb2": gb2.astype(np.float32),
        "wrow": wrow,
    }
    in_maps = []
    xf = np.asarray(x).reshape(2, F1, V, 4096)
    for core in range(N_CORES):
        b, q = core // 4, core % 4
        xs = xf[b, :, :, q * S:(q + 1) * S]            # [16, 162, 1024]
        # xk2: [V, NG, F1*SB]
        xkc = xs.transpose(1, 0, 2).reshape(V, F1, NG, SB)
        xkc = xkc.transpose(0, 2, 1, 3).reshape(V, NG, F1 * SB)
        # brs1: [NG, 80, (u 168, s 256)]; rows 0:16 = k0 = x itself
        br = np.zeros((NG, 80, UP, SB), np.float32)
        br[:, 0:F1, 0:V, :] = xs.reshape(F1, V, NG, SB).transpose(2, 0, 1, 3)
        m = dict(common)
        m["xk2"] = np.ascontiguousarray(xkc).astype(BF16)
        m["brs1"] = np.ascontiguousarray(
            br.reshape(NG, 80, ROWC)).astype(BF16)
        in_maps.append(m)
    return in_maps


_CACHE = {}


def _run(in_maps, trace=False):
    if "nc" not in _CACHE:
        _CACHE["nc"] = build_program()
    return run_bass_kernel_spmd(
        _CACHE["nc"], in_maps, core_ids=list(range(N_CORES)), trace=trace)


def kernel(x, lap, w1, b1, g1, be1, w2, b2, g2, be2, _trace=False):
    # conv biases b1/b2 cancel exactly inside BatchNorm -> ignored
    in_maps = _host_prep(x, lap, w1, w2, g1, be1, g2, be2)
    res = _run(in_maps, trace=_trace)
    _CACHE["last_results"] = res
    full = np.empty((2, F2, V, 4096), np.float32)
    for core in range(N_CORES):
        b, q = core // 4, core % 4
        full[b, :, :, q * S:(q + 1) * S] = \
            res.results[core]["out"].astype(np.float32)
    return full.reshape(2, F2, V, 16, 16, 16)
